# revision 5
# baseline (speedup 1.0000x reference)
"""Trainium2 Bass kernel for nn_CosineProxy.

Reference computation (per task b):
    feats[n]  = blockmean_pool(x[b,n])            # (640,10,10) -> 800 dims
    proxy     = sum_n feats[n]                     # pooling is linear
    sim[n]    = <feats[n], proxy> / max(||feats[n]||*||proxy||, eps)
    out[b]    = sum_n sim[n] * x[b,n]

sim is scale-invariant, so block-SUM pooling is used instead of block-mean.
Sharding: pure data parallelism over B=256 tasks -> 32 tasks per core x 8 cores.

Per-core layout: x[b,n] (640*100 contiguous floats) lives in SBUF as
(128 partitions, 500 free) where partition p holds channels [5p,5p+5).
A 20-channel pooling block == 4 partitions x 5 in-partition channels.

Pipeline per group of 4 tasks (engine-balanced; fp32 matmul costs 4
cycles/row on TRN2 PE so the PE only sees spatially-pooled data + the
output accumulation):
  1. DVE: 2x2 spatial pooling as two whole-task strided tensor_tensor
     adds: (128, 2500) -> (128, 625) per task.
  2. PE: "packing" matmuls (lhsT = block-indicator) channel-pool 4 tasks
     into PSUM (125 cols per shot); DVE strided reduce finishes the
     in-partition channel sum -> pooled feats F4 + proxy P4.
  3. DVE: two fused product maps + reduces -> per-(task,oc) partial Gram
     terms QS; PE ones-block matmuls reduce across each task's 32
     partitions and broadcast to all 128; small ops -> cosine sims simt.
  4. Weighted shot sum: ACT scales shots 0-2 (per-partition scalar
     multiply), PE accumulates them in PSUM via identity matmuls; shots
     3-4 fold in as fused multiply-adds on DVE and GPSIMD; DMA out.
"""

import numpy as np

import concourse.bacc as bacc
import concourse.mybir as mybir
import concourse.tile as tile
from concourse.bass_utils import run_bass_kernel_spmd

F32 = mybir.dt.float32
ADD = mybir.AluOpType.add
MULT = mybir.AluOpType.mult

P = 128          # SBUF partitions
N = 5            # shots
C = 640          # channels
HW = 100         # 10*10 spatial
CF = C // P      # 5 channels per partition
FREE = CF * HW   # 500 floats per partition per (b, n)
OS = 25          # pooled spatial size (5*5)
SF = CF * OS     # 125: spatially-pooled cols per (b, n)
EPS = 1e-8
NCORES = 8
B = 256
BC = B // NCORES  # 32 tasks per core


def consts_np() -> np.ndarray:
    """(128, 1152) constant matrix: 4 packing mats, 4 ones-blocks, identity."""
    cs = np.zeros((P, 1152), np.float32)
    for t in range(4):
        for p in range(P):
            # B4t: route channel-partition p of task t to oc row t*32 + p//4
            cs[p, t * 128 + t * 32 + p // 4] = 1.0
        # OBt: ones on rows [32t, 32t+32), all 128 output columns
        cs[32 * t:32 * (t + 1), 512 + t * 128: 512 + (t + 1) * 128] = 1.0
    cs[np.arange(P), 1024 + np.arange(P)] = 1.0  # identity
    return cs


def build(bc: int = BC, reps: int = 1):
    """Build + compile the per-core Bass module for a bc-task shard."""
    assert bc % 4 == 0
    nc = bacc.Bacc("TRN2", target_bir_lowering=False, debug=False,
                   num_devices=NCORES)
    x_in = nc.dram_tensor("x", (bc, N, C, HW), F32, kind="ExternalInput")
    cs_in = nc.dram_tensor("consts", (P, 1152), F32, kind="ExternalInput")
    out_d = nc.dram_tensor("out", (bc, C, HW), F32, kind="ExternalOutput")

    xv = x_in[:].rearrange("b n (p cf) hw -> b p n (cf hw)", p=P, cf=CF)
    ov = out_d[:].rearrange("b (p cf) hw -> b p (cf hw)", p=P, cf=CF)

    with tile.TileContext(nc) as tc:
        with (
            tc.tile_pool(name="cpool", bufs=1) as cpool,
            tc.tile_pool(name="xpool", bufs=8) as xpool,
            tc.tile_pool(name="wpool", bufs=2) as wpool,
            tc.tile_pool(name="s2pool", bufs=8) as s2pool,
            tc.tile_pool(name="spool", bufs=2) as spool,
            tc.tile_pool(name="pkpool", bufs=2, space="PSUM") as pkpool,
            tc.tile_pool(name="rdpool", bufs=2, space="PSUM") as rdpool,
        ):
            cs = cpool.tile([P, 1152], F32)
            nc.sync.dma_start(cs[:], cs_in[:])
            lhs_pack = [cs[:, t * 128:(t + 1) * 128] for t in range(4)]
            lhs_ones = [cs[:, 512 + t * 128:512 + (t + 1) * 128]
                        for t in range(4)]
            lhs_eye = cs[:, 1024:1152]

            for g in range(reps * (bc // 4)):
                g = g % (bc // 4)
                xts, s2ts = [], []
                for t in range(4):
                    xt = xpool.tile([P, N, FREE], F32, tag="x")
                    nc.sync.dma_start(xt[:], xv[4 * g + t])
                    xts.append(xt)
                    # 2x2 spatial pooling, whole task at once
                    s1 = wpool.tile([P, N * CF * 50], F32, tag="s1")
                    v = xt[:].rearrange("p n (ci h wo dw) -> p (n ci) h wo dw",
                                        ci=CF, h=10, wo=5, dw=2)
                    nc.vector.tensor_tensor(
                        out=s1[:].rearrange("p (a h wo) -> p a h wo",
                                            a=N * CF, wo=5),
                        in0=v[:, :, :, :, 0], in1=v[:, :, :, :, 1], op=ADD)
                    s2 = s2pool.tile([P, N * SF], F32, tag="s2")
                    v1 = s1[:].rearrange("p (a ho dh wo) -> p a ho dh wo",
                                         a=N * CF, ho=5, dh=2)
                    nc.vector.tensor_tensor(
                        out=s2[:].rearrange("p (a ho wo) -> p a ho wo",
                                            a=N * CF, wo=5),
                        in0=v1[:, :, :, 0, :], in1=v1[:, :, :, 1, :], op=ADD)
                    s2ts.append(s2)

                # --- channel pooling: pack 4 tasks into PSUM, 2 banks ---
                pkA = pkpool.tile([P, 3 * SF], F32, tag="pk")
                pkB = pkpool.tile([P, 2 * SF], F32, tag="pk")
                for n in range(N):
                    pk = pkA[:, n * SF:(n + 1) * SF] if n < 3 else \
                        pkB[:, (n - 3) * SF:(n - 2) * SF]
                    for t in range(4):
                        nc.tensor.matmul(pk, lhs_pack[t],
                                         s2ts[t][:, n * SF:(n + 1) * SF],
                                         start=(t == 0), stop=(t == 3))
                # FP: pooled feats [n0..n4] then proxy P at cols 125:150
                FP = spool.tile([P, 6 * OS], F32, tag="FP")
                nc.vector.tensor_reduce(
                    out=FP[:, 0:3 * OS],
                    in_=pkA[:].rearrange("p (j ci s) -> p j s ci", j=3, ci=CF),
                    axis=mybir.AxisListType.X, op=ADD)
                nc.vector.tensor_reduce(
                    out=FP[:, 3 * OS:5 * OS],
                    in_=pkB[:].rearrange("p (j ci s) -> p j s ci", j=2, ci=CF),
                    axis=mybir.AxisListType.X, op=ADD)
                nc.vector.tensor_reduce(
                    out=FP[:, 5 * OS:6 * OS],
                    in_=FP[:, 0:5 * OS].rearrange("p (n s) -> p s n", n=N),
                    axis=mybir.AxisListType.X, op=ADD)

                # --- Gram terms. QS cols: 0..4 <F_n,P>, 5 <P,P>, 6..10 <F_n,F_n>
                QP = spool.tile([P, 11 * OS], F32, tag="QP")
                nc.vector.tensor_tensor(
                    out=QP[:, 0:6 * OS].rearrange("p (b s) -> p b s", b=6),
                    in0=FP[:].rearrange("p (b s) -> p b s", b=6),
                    in1=FP[:, 5 * OS:6 * OS].rearrange(
                        "p (b s) -> p b s", b=1).broadcast_to((P, 6, OS)),
                    op=MULT)
                nc.vector.tensor_tensor(
                    out=QP[:, 6 * OS:11 * OS], in0=FP[:, 0:5 * OS],
                    in1=FP[:, 0:5 * OS], op=MULT)
                QS = spool.tile([P, 11], F32, tag="QS")
                nc.vector.tensor_reduce(
                    out=QS[:], in_=QP[:].rearrange("p (q s) -> p q s", q=11),
                    axis=mybir.AxisListType.X, op=ADD)

                # --- cross-partition reduce + broadcast to all partitions ---
                rd = rdpool.tile([P, 44], F32, tag="rd")
                for t in range(4):
                    nc.tensor.matmul(rd[:, t * 11:(t + 1) * 11], lhs_ones[t],
                                     QS[:], start=True, stop=True)
                rsb = spool.tile([P, 44], F32, tag="rsb")
                nc.vector.tensor_copy(rsb[:], rd[:])
                rv = rsb[:].rearrange("p (t q) -> p t q", t=4)

                # --- cosine sims: sim = dot / max(sqrt(na2*nb2), eps) ---
                prod = spool.tile([P, 20], F32, tag="prod")
                nc.vector.tensor_tensor(
                    out=prod[:].rearrange("p (t n) -> p t n", t=4),
                    in0=rv[:, :, 6:11],
                    in1=rv[:, :, 5:6].broadcast_to((P, 4, 5)), op=MULT)
                sq = spool.tile([P, 20], F32, tag="sq")
                nc.scalar.activation(sq[:], prod[:],
                                     mybir.ActivationFunctionType.Sqrt)
                mx = spool.tile([P, 20], F32, tag="mx")
                nc.vector.tensor_scalar_max(mx[:], sq[:], EPS)
                rs = spool.tile([P, 20], F32, tag="rs")
                nc.vector.reciprocal(rs[:], mx[:])
                simt = spool.tile([P, 20], F32, tag="simt")
                nc.vector.tensor_tensor(
                    out=simt[:].rearrange("p (t n) -> p t n", t=4),
                    in0=rv[:, :, 0:5],
                    in1=rs[:].rearrange("p (t n) -> p t n", t=4), op=MULT)

                # --- weighted sum of raw shots, spread across ACT/DVE/GP:
                # ACT scales shots 0/2/4, GP scales shot 3, DVE folds shot 1
                # (fused scale+add), GP does the remaining three adds ---
                for t in range(4):
                    def st(n):
                        return simt[:, t * 5 + n:t * 5 + n + 1]
                    a0 = wpool.tile([P, FREE], F32, tag="a0")
                    nc.scalar.activation(
                        a0[:], xts[t][:, 0, :],
                        mybir.ActivationFunctionType.Copy, scale=st(0))
                    a2 = wpool.tile([P, FREE], F32, tag="a2")
                    nc.scalar.activation(
                        a2[:], xts[t][:, 2, :],
                        mybir.ActivationFunctionType.Copy, scale=st(2))
                    a4 = wpool.tile([P, FREE], F32, tag="a4")
                    nc.scalar.activation(
                        a4[:], xts[t][:, 4, :],
                        mybir.ActivationFunctionType.Copy, scale=st(4))
                    a3 = wpool.tile([P, FREE], F32, tag="a3")
                    nc.gpsimd.tensor_scalar(
                        out=a3[:], in0=xts[t][:, 3, :], scalar1=st(3),
                        scalar2=None, op0=MULT)
                    e1 = wpool.tile([P, FREE], F32, tag="e1")
                    nc.vector.scalar_tensor_tensor(
                        out=e1[:], in0=xts[t][:, 1, :], scalar=st(1),
                        in1=a0[:], op0=MULT, op1=ADD)
                    m2 = wpool.tile([P, FREE], F32, tag="m2")
                    nc.gpsimd.tensor_tensor(
                        out=m2[:], in0=a3[:], in1=a4[:], op=ADD)
                    m1 = wpool.tile([P, FREE], F32, tag="m1")
                    nc.gpsimd.tensor_tensor(
                        out=m1[:], in0=e1[:], in1=a2[:], op=ADD)
                    ob = wpool.tile([P, FREE], F32, tag="ob")
                    nc.gpsimd.tensor_tensor(
                        out=ob[:], in0=m1[:], in1=m2[:], op=ADD)
                    nc.sync.dma_start(ov[4 * g + t], ob[:])

    nc.compile()
    return nc


_CACHE = {}


def _get_nc(bc: int = BC):
    if bc not in _CACHE:
        _CACHE[bc] = build(bc)
    return _CACHE[bc]


def kernel(x: np.ndarray) -> np.ndarray:
    assert x.shape == (B, N, C, 10, 10) and x.dtype == np.float32
    nc = _get_nc(BC)
    cs = consts_np()
    shards = np.ascontiguousarray(x.reshape(NCORES, BC, N, C, HW))
    in_maps = [{"x": shards[i], "consts": cs} for i in range(NCORES)]
    res = run_bass_kernel_spmd(nc, in_maps, core_ids=list(range(NCORES)))
    out = np.concatenate([res.results[i]["out"] for i in range(NCORES)])
    return out.reshape(B, C, 10, 10).astype(np.float32)



# revision 10
# speedup vs baseline: 1.9796x; 1.9796x over previous
"""Trainium2 Bass kernel for nn_CosineProxy.

Reference computation (per task b):
    feats[n]  = blockmean_pool(x[b,n])            # (640,10,10) -> 800 dims
    proxy     = sum_n feats[n]                     # pooling is linear
    sim[n]    = <feats[n], proxy> / max(||feats[n]||*||proxy||, eps)
    out[b]    = sum_n sim[n] * x[b,n]

sim is scale-invariant, so block-SUM pooling is used instead of block-mean.
Sharding: pure data parallelism over B=256 tasks -> 32 tasks per core x 8 cores.

Per-core layout: x[b,n] (640*100 contiguous floats) lives in SBUF as
(128 partitions, 500 free) where partition p holds channels [5p,5p+5).
A 20-channel pooling block == 4 partitions x 5 in-partition channels.

Pipeline per group of 4 tasks (engine-balanced; fp32 matmul costs 4
cycles/row on TRN2 PE so the PE only sees spatially-pooled data + the
output accumulation):
  1. DVE: 2x2 spatial pooling as two whole-task strided tensor_tensor
     adds: (128, 2500) -> (128, 625) per task.
  2. PE: "packing" matmuls (lhsT = block-indicator) channel-pool 4 tasks
     into PSUM (125 cols per shot); DVE strided reduce finishes the
     in-partition channel sum -> pooled feats F4 + proxy P4.
  3. DVE: two fused product maps + reduces -> per-(task,oc) partial Gram
     terms QS; PE ones-block matmuls reduce across each task's 32
     partitions and broadcast to all 128; small ops -> cosine sims simt.
  4. Weighted shot sum: ACT scales shots 0-2 (per-partition scalar
     multiply), PE accumulates them in PSUM via identity matmuls; shots
     3-4 fold in as fused multiply-adds on DVE and GPSIMD; DMA out.
"""

import numpy as np

import concourse.bacc as bacc
import concourse.mybir as mybir
import concourse.tile as tile
from concourse.bass_utils import run_bass_kernel_spmd

F32 = mybir.dt.float32
BF16 = mybir.dt.bfloat16
ADD = mybir.AluOpType.add
MULT = mybir.AluOpType.mult

P = 128          # SBUF partitions
N = 5            # shots
C = 640          # channels
HW = 100         # 10*10 spatial
CF = C // P      # 5 channels per partition
FREE = CF * HW   # 500 floats per partition per (b, n)
OS = 25          # pooled spatial size (5*5)
SF = CF * OS     # 125: spatially-pooled cols per (b, n)
EPS = 1e-8
NCORES = 8
B = 256
BC = B // NCORES  # 32 tasks per core


def consts_np() -> np.ndarray:
    """(128, 1152) constant matrix: 4 packing mats, 4 ones-blocks, identity."""
    cs = np.zeros((P, 1152), np.float32)
    for t in range(4):
        for p in range(P):
            # B4t: route channel-partition p of task t to oc row t*32 + p//4
            cs[p, t * 128 + t * 32 + p // 4] = 1.0
        # OBt: ones on rows [32t, 32t+32), all 128 output columns
        cs[32 * t:32 * (t + 1), 512 + t * 128: 512 + (t + 1) * 128] = 1.0
    cs[np.arange(P), 1024 + np.arange(P)] = 1.0  # identity
    return cs


def build(bc: int = BC, reps: int = 1):
    """Build + compile the per-core Bass module for a bc-task shard."""
    assert bc % 4 == 0
    nc = bacc.Bacc("TRN2", target_bir_lowering=False, debug=False,
                   num_devices=NCORES)
    x_in = nc.dram_tensor("x", (bc, N, C, HW), F32, kind="ExternalInput")
    cs_in = nc.dram_tensor("consts", (P, 1152), F32, kind="ExternalInput")
    out_d = nc.dram_tensor("out", (bc, C, HW), F32, kind="ExternalOutput")

    xv = x_in[:].rearrange("b n (p cf) hw -> b p n (cf hw)", p=P, cf=CF)
    ov = out_d[:].rearrange("b (p cf) hw -> b p (cf hw)", p=P, cf=CF)

    with tile.TileContext(nc) as tc:
        with (
            tc.tile_pool(name="cpool", bufs=1) as cpool,
            tc.tile_pool(name="xpool", bufs=8) as xpool,
            tc.tile_pool(name="wpool", bufs=2) as wpool,
            tc.tile_pool(name="s2pool", bufs=8) as s2pool,
            tc.tile_pool(name="spool", bufs=2) as spool,
            tc.tile_pool(name="pkpool", bufs=2, space="PSUM") as pkpool,
            tc.tile_pool(name="rdpool", bufs=2, space="PSUM") as rdpool,
            tc.tile_pool(name="eapool", bufs=3, space="PSUM") as eapool,
        ):
            cs = cpool.tile([P, 1152], F32)
            nc.sync.dma_start(cs[:], cs_in[:])
            # bf16 copy of the pack matrices (0/1 -> exact in bf16): the
            # pack matmuls then run at 1 cycle/row instead of 4.
            csb = cpool.tile([P, 512], BF16)
            nc.vector.tensor_copy(csb[:], cs[:, 0:512])
            lhs_pack = [csb[:, t * 128:(t + 1) * 128] for t in range(4)]
            lhs_ones = [cs[:, 512 + t * 128:512 + (t + 1) * 128]
                        for t in range(4)]
            lhs_eye = cs[:, 1024:1152]

            for g in range(reps * (bc // 4)):
                g = g % (bc // 4)
                xts, s2ts = [], []
                for t in range(4):
                    xt = xpool.tile([P, N, FREE], F32, tag="x")
                    nc.sync.dma_start(xt[:], xv[4 * g + t])
                    xts.append(xt)
                    # 2x2 spatial pooling, whole task at once; bf16 from
                    # stage 1 on (pooled feats only feed the cosine sims,
                    # tolerance is loose), so stage 2 runs in DVE 2x mode
                    # and the pack matmuls take bf16 rhs.
                    s1 = wpool.tile([P, N * CF * 50], BF16, tag="s1")
                    v = xt[:].rearrange("p n (ci h wo dw) -> p (n ci) h wo dw",
                                        ci=CF, h=10, wo=5, dw=2)
                    nc.vector.tensor_tensor(
                        out=s1[:].rearrange("p (a h wo) -> p a h wo",
                                            a=N * CF, wo=5),
                        in0=v[:, :, :, :, 0], in1=v[:, :, :, :, 1], op=ADD)
                    s2 = s2pool.tile([P, N * SF], BF16, tag="s2")
                    v1 = s1[:].rearrange("p (a ho dh wo) -> p a ho dh wo",
                                         a=N * CF, ho=5, dh=2)
                    nc.vector.tensor_tensor(
                        out=s2[:].rearrange("p (a ho wo) -> p a ho wo",
                                            a=N * CF, wo=5),
                        in0=v1[:, :, :, 0, :], in1=v1[:, :, :, 1, :], op=ADD)
                    s2ts.append(s2)

                # --- channel pooling: pack 4 tasks into PSUM, 2 banks ---
                pkA = pkpool.tile([P, 3 * SF], F32, tag="pk")
                pkB = pkpool.tile([P, 2 * SF], F32, tag="pk")
                for n in range(N):
                    pk = pkA[:, n * SF:(n + 1) * SF] if n < 3 else \
                        pkB[:, (n - 3) * SF:(n - 2) * SF]
                    for t in range(4):
                        nc.tensor.matmul(pk, lhs_pack[t],
                                         s2ts[t][:, n * SF:(n + 1) * SF],
                                         start=(t == 0), stop=(t == 3))
                # FP: pooled feats [n0..n4] then proxy P at cols 125:150
                FP = spool.tile([P, 6 * OS], F32, tag="FP")
                nc.vector.tensor_reduce(
                    out=FP[:, 0:3 * OS],
                    in_=pkA[:].rearrange("p (j ci s) -> p j s ci", j=3, ci=CF),
                    axis=mybir.AxisListType.X, op=ADD)
                nc.vector.tensor_reduce(
                    out=FP[:, 3 * OS:5 * OS],
                    in_=pkB[:].rearrange("p (j ci s) -> p j s ci", j=2, ci=CF),
                    axis=mybir.AxisListType.X, op=ADD)
                nc.vector.tensor_reduce(
                    out=FP[:, 5 * OS:6 * OS],
                    in_=FP[:, 0:5 * OS].rearrange("p (n s) -> p s n", n=N),
                    axis=mybir.AxisListType.X, op=ADD)

                # --- Gram terms. QS cols: 0..4 <F_n,P>, 5 <P,P>, 6..10 <F_n,F_n>
                QP = spool.tile([P, 11 * OS], F32, tag="QP")
                nc.vector.tensor_tensor(
                    out=QP[:, 0:6 * OS].rearrange("p (b s) -> p b s", b=6),
                    in0=FP[:].rearrange("p (b s) -> p b s", b=6),
                    in1=FP[:, 5 * OS:6 * OS].rearrange(
                        "p (b s) -> p b s", b=1).broadcast_to((P, 6, OS)),
                    op=MULT)
                nc.vector.tensor_tensor(
                    out=QP[:, 6 * OS:11 * OS], in0=FP[:, 0:5 * OS],
                    in1=FP[:, 0:5 * OS], op=MULT)
                QS = spool.tile([P, 11], F32, tag="QS")
                nc.vector.tensor_reduce(
                    out=QS[:], in_=QP[:].rearrange("p (q s) -> p q s", q=11),
                    axis=mybir.AxisListType.X, op=ADD)

                # --- cross-partition reduce + broadcast to all partitions ---
                rd = rdpool.tile([P, 44], F32, tag="rd")
                for t in range(4):
                    nc.tensor.matmul(rd[:, t * 11:(t + 1) * 11], lhs_ones[t],
                                     QS[:], start=True, stop=True)
                rsb = spool.tile([P, 44], F32, tag="rsb")
                nc.vector.tensor_copy(rsb[:], rd[:])
                rv = rsb[:].rearrange("p (t q) -> p t q", t=4)

                # --- cosine sims: sim = dot / max(sqrt(na2*nb2), eps) ---
                prod = spool.tile([P, 20], F32, tag="prod")
                nc.vector.tensor_tensor(
                    out=prod[:].rearrange("p (t n) -> p t n", t=4),
                    in0=rv[:, :, 6:11],
                    in1=rv[:, :, 5:6].broadcast_to((P, 4, 5)), op=MULT)
                sq = spool.tile([P, 20], F32, tag="sq")
                nc.scalar.activation(sq[:], prod[:],
                                     mybir.ActivationFunctionType.Sqrt)
                mx = spool.tile([P, 20], F32, tag="mx")
                nc.vector.tensor_scalar_max(mx[:], sq[:], EPS)
                rs = spool.tile([P, 20], F32, tag="rs")
                nc.vector.reciprocal(rs[:], mx[:])
                simt = spool.tile([P, 20], F32, tag="simt")
                nc.vector.tensor_tensor(
                    out=simt[:].rearrange("p (t n) -> p t n", t=4),
                    in0=rv[:, :, 0:5],
                    in1=rs[:].rearrange("p (t n) -> p t n", t=4), op=MULT)

                # --- weighted sum of raw shots. ACT scales shots 0/2/4 and
                # DVE (2x tensor_scalar) shot 3; PE accumulates those four
                # in PSUM via identity matmuls; DVE folds shot 1 with a
                # fused scale+add reading the PSUM partial ---
                for t in range(4):
                    def st(n):
                        return simt[:, t * 5 + n:t * 5 + n + 1]
                    a0 = wpool.tile([P, FREE], F32, tag="a0")
                    nc.scalar.activation(
                        a0[:], xts[t][:, 0, :],
                        mybir.ActivationFunctionType.Copy, scale=st(0))
                    a2 = wpool.tile([P, FREE], F32, tag="a2")
                    nc.scalar.activation(
                        a2[:], xts[t][:, 2, :],
                        mybir.ActivationFunctionType.Copy, scale=st(2))
                    a4 = wpool.tile([P, FREE], F32, tag="a4")
                    nc.scalar.activation(
                        a4[:], xts[t][:, 4, :],
                        mybir.ActivationFunctionType.Copy, scale=st(4))
                    a3 = wpool.tile([P, FREE], F32, tag="a3")
                    nc.vector.tensor_scalar(
                        out=a3[:], in0=xts[t][:, 3, :], scalar1=st(3),
                        scalar2=None, op0=MULT)
                    ea = eapool.tile([P, FREE], F32, tag="ea")
                    nc.tensor.matmul(ea[:], lhs_eye, a0[:],
                                     start=True, stop=False)
                    nc.tensor.matmul(ea[:], lhs_eye, a2[:],
                                     start=False, stop=False)
                    nc.tensor.matmul(ea[:], lhs_eye, a4[:],
                                     start=False, stop=False)
                    nc.tensor.matmul(ea[:], lhs_eye, a3[:],
                                     start=False, stop=True)
                    ob = wpool.tile([P, FREE], F32, tag="ob")
                    nc.vector.scalar_tensor_tensor(
                        out=ob[:], in0=xts[t][:, 1, :], scalar=st(1),
                        in1=ea[:], op0=MULT, op1=ADD)
                    nc.sync.dma_start(ov[4 * g + t], ob[:])

    nc.compile()
    return nc


_CACHE = {}


def _get_nc(bc: int = BC):
    if bc not in _CACHE:
        _CACHE[bc] = build(bc)
    return _CACHE[bc]


def kernel(x: np.ndarray) -> np.ndarray:
    assert x.shape == (B, N, C, 10, 10) and x.dtype == np.float32
    nc = _get_nc(BC)
    cs = consts_np()
    shards = np.ascontiguousarray(x.reshape(NCORES, BC, N, C, HW))
    in_maps = [{"x": shards[i], "consts": cs} for i in range(NCORES)]
    res = run_bass_kernel_spmd(nc, in_maps, core_ids=list(range(NCORES)))
    out = np.concatenate([res.results[i]["out"] for i in range(NCORES)])
    return out.reshape(B, C, 10, 10).astype(np.float32)



# revision 12
# speedup vs baseline: 2.2521x; 1.1376x over previous
"""Trainium2 Bass kernel for nn_CosineProxy.

Reference computation (per task b):
    feats[n]  = blockmean_pool(x[b,n])            # (640,10,10) -> 800 dims
    proxy     = sum_n feats[n]                     # pooling is linear
    sim[n]    = <feats[n], proxy> / max(||feats[n]||*||proxy||, eps)
    out[b]    = sum_n sim[n] * x[b,n]

sim is scale-invariant, so block-SUM pooling is used instead of block-mean.
Sharding: pure data parallelism over B=256 tasks -> 32 tasks per core x 8 cores.

Per-core layout: x[b,n] (640*100 contiguous floats) lives in SBUF as
(128 partitions, 500 free) where partition p holds channels [5p,5p+5).
A 20-channel pooling block == 4 partitions x 5 in-partition channels.

Pipeline per group of 4 tasks (engine-balanced; fp32 matmul costs 4
cycles/row on TRN2 PE so the PE only sees spatially-pooled data + the
output accumulation):
  1. DVE: 2x2 spatial pooling as two whole-task strided tensor_tensor
     adds: (128, 2500) -> (128, 625) per task.
  2. PE: "packing" matmuls (lhsT = block-indicator) channel-pool 4 tasks
     into PSUM (125 cols per shot); DVE strided reduce finishes the
     in-partition channel sum -> pooled feats F4 + proxy P4.
  3. DVE: two fused product maps + reduces -> per-(task,oc) partial Gram
     terms QS; PE ones-block matmuls reduce across each task's 32
     partitions and broadcast to all 128; small ops -> cosine sims simt.
  4. Weighted shot sum: ACT scales shots 0-2 (per-partition scalar
     multiply), PE accumulates them in PSUM via identity matmuls; shots
     3-4 fold in as fused multiply-adds on DVE and GPSIMD; DMA out.
"""

import numpy as np

import concourse.bacc as bacc
import concourse.mybir as mybir
import concourse.tile as tile
from concourse.bass_utils import run_bass_kernel_spmd

F32 = mybir.dt.float32
BF16 = mybir.dt.bfloat16
ADD = mybir.AluOpType.add
MULT = mybir.AluOpType.mult

P = 128          # SBUF partitions
N = 5            # shots
C = 640          # channels
HW = 100         # 10*10 spatial
CF = C // P      # 5 channels per partition
FREE = CF * HW   # 500 floats per partition per (b, n)
OS = 25          # pooled spatial size (5*5)
SF = CF * OS     # 125: spatially-pooled cols per (b, n)
EPS = 1e-8
NCORES = 8
B = 256
BC = B // NCORES  # 32 tasks per core


def consts_np() -> np.ndarray:
    """(128, 1152) constant matrix: 4 packing mats, 4 ones-blocks, identity."""
    cs = np.zeros((P, 1152), np.float32)
    for t in range(4):
        for p in range(P):
            # B4t: route channel-partition p of task t to oc row t*32 + p//4
            cs[p, t * 128 + t * 32 + p // 4] = 1.0
        # OBt: ones on rows [32t, 32t+32), all 128 output columns
        cs[32 * t:32 * (t + 1), 512 + t * 128: 512 + (t + 1) * 128] = 1.0
    cs[np.arange(P), 1024 + np.arange(P)] = 1.0  # identity
    return cs


def build(bc: int = BC, reps: int = 1):
    """Build + compile the per-core Bass module for a bc-task shard."""
    assert bc % 4 == 0
    nc = bacc.Bacc("TRN2", target_bir_lowering=False, debug=False,
                   num_devices=NCORES)
    x_in = nc.dram_tensor("x", (bc, N, C, HW), F32, kind="ExternalInput")
    cs_in = nc.dram_tensor("consts", (P, 1152), F32, kind="ExternalInput")
    out_d = nc.dram_tensor("out", (bc, C, HW), F32, kind="ExternalOutput")

    xv = x_in[:].rearrange("b n (p cf) hw -> b p n (cf hw)", p=P, cf=CF)
    ov = out_d[:].rearrange("b (p cf) hw -> b p (cf hw)", p=P, cf=CF)

    with tile.TileContext(nc) as tc:
        with (
            tc.tile_pool(name="cpool", bufs=1) as cpool,
            tc.tile_pool(name="xpool", bufs=8) as xpool,
            tc.tile_pool(name="wpool", bufs=2) as wpool,
            tc.tile_pool(name="s2pool", bufs=8) as s2pool,
            tc.tile_pool(name="spool", bufs=2) as spool,
            tc.tile_pool(name="pkpool", bufs=2, space="PSUM") as pkpool,
            tc.tile_pool(name="rdpool", bufs=2, space="PSUM") as rdpool,
            tc.tile_pool(name="eapool", bufs=3, space="PSUM") as eapool,
        ):
            cs = cpool.tile([P, 1152], F32)
            nc.sync.dma_start(cs[:], cs_in[:])
            # bf16 copy of the pack/eye matrices (0/1 -> exact in bf16):
            # those matmuls then run at 1 cycle/row instead of 4.
            csb = cpool.tile([P, 1152], BF16)
            nc.vector.tensor_copy(csb[:], cs[:])
            lhs_pack = [csb[:, t * 128:(t + 1) * 128] for t in range(4)]
            lhs_ones = [cs[:, 512 + t * 128:512 + (t + 1) * 128]
                        for t in range(4)]
            lhs_eye = csb[:, 1024:1152]

            for g in range(reps * (bc // 4)):
                g = g % (bc // 4)
                xts, s2ts = [], []
                for t in range(4):
                    xt = xpool.tile([P, N, FREE], F32, tag="x")
                    nc.sync.dma_start(xt[:], xv[4 * g + t])
                    xts.append(xt)
                    # 2x2 spatial pooling, whole task at once; bf16 from
                    # stage 1 on (pooled feats only feed the cosine sims,
                    # tolerance is loose), so stage 2 runs in DVE 2x mode
                    # and the pack matmuls take bf16 rhs.
                    s1 = wpool.tile([P, N * CF * 50], BF16, tag="s1")
                    v = xt[:].rearrange("p n (ci h wo dw) -> p (n ci) h wo dw",
                                        ci=CF, h=10, wo=5, dw=2)
                    nc.vector.tensor_tensor(
                        out=s1[:].rearrange("p (a h wo) -> p a h wo",
                                            a=N * CF, wo=5),
                        in0=v[:, :, :, :, 0], in1=v[:, :, :, :, 1], op=ADD)
                    s2 = s2pool.tile([P, N * SF], BF16, tag="s2")
                    v1 = s1[:].rearrange("p (a ho dh wo) -> p a ho dh wo",
                                         a=N * CF, ho=5, dh=2)
                    nc.vector.tensor_tensor(
                        out=s2[:].rearrange("p (a ho wo) -> p a ho wo",
                                            a=N * CF, wo=5),
                        in0=v1[:, :, :, 0, :], in1=v1[:, :, :, 1, :], op=ADD)
                    s2ts.append(s2)

                # --- channel pooling: pack 4 tasks into PSUM, 2 banks ---
                pkA = pkpool.tile([P, 3 * SF], F32, tag="pk")
                pkB = pkpool.tile([P, 2 * SF], F32, tag="pk")
                for n in range(N):
                    pk = pkA[:, n * SF:(n + 1) * SF] if n < 3 else \
                        pkB[:, (n - 3) * SF:(n - 2) * SF]
                    for t in range(4):
                        nc.tensor.matmul(pk, lhs_pack[t],
                                         s2ts[t][:, n * SF:(n + 1) * SF],
                                         start=(t == 0), stop=(t == 3))
                # FP: pooled feats [n0..n4] then proxy P at cols 125:150
                FP = spool.tile([P, 6 * OS], F32, tag="FP")
                nc.vector.tensor_reduce(
                    out=FP[:, 0:3 * OS],
                    in_=pkA[:].rearrange("p (j ci s) -> p j s ci", j=3, ci=CF),
                    axis=mybir.AxisListType.X, op=ADD)
                nc.vector.tensor_reduce(
                    out=FP[:, 3 * OS:5 * OS],
                    in_=pkB[:].rearrange("p (j ci s) -> p j s ci", j=2, ci=CF),
                    axis=mybir.AxisListType.X, op=ADD)
                nc.vector.tensor_reduce(
                    out=FP[:, 5 * OS:6 * OS],
                    in_=FP[:, 0:5 * OS].rearrange("p (n s) -> p s n", n=N),
                    axis=mybir.AxisListType.X, op=ADD)

                # --- Gram terms. QS cols: 0..4 <F_n,P>, 5 <P,P>, 6..10 <F_n,F_n>
                QP = spool.tile([P, 11 * OS], F32, tag="QP")
                nc.vector.tensor_tensor(
                    out=QP[:, 0:6 * OS].rearrange("p (b s) -> p b s", b=6),
                    in0=FP[:].rearrange("p (b s) -> p b s", b=6),
                    in1=FP[:, 5 * OS:6 * OS].rearrange(
                        "p (b s) -> p b s", b=1).broadcast_to((P, 6, OS)),
                    op=MULT)
                nc.vector.tensor_tensor(
                    out=QP[:, 6 * OS:11 * OS], in0=FP[:, 0:5 * OS],
                    in1=FP[:, 0:5 * OS], op=MULT)
                QS = spool.tile([P, 11], F32, tag="QS")
                nc.vector.tensor_reduce(
                    out=QS[:], in_=QP[:].rearrange("p (q s) -> p q s", q=11),
                    axis=mybir.AxisListType.X, op=ADD)

                # --- cross-partition reduce + broadcast to all partitions ---
                rd = rdpool.tile([P, 44], F32, tag="rd")
                for t in range(4):
                    nc.tensor.matmul(rd[:, t * 11:(t + 1) * 11], lhs_ones[t],
                                     QS[:], start=True, stop=True)
                rsb = spool.tile([P, 44], F32, tag="rsb")
                nc.vector.tensor_copy(rsb[:], rd[:])
                rv = rsb[:].rearrange("p (t q) -> p t q", t=4)

                # --- cosine sims: sim = dot / max(sqrt(na2*nb2), eps) ---
                prod = spool.tile([P, 20], F32, tag="prod")
                nc.vector.tensor_tensor(
                    out=prod[:].rearrange("p (t n) -> p t n", t=4),
                    in0=rv[:, :, 6:11],
                    in1=rv[:, :, 5:6].broadcast_to((P, 4, 5)), op=MULT)
                sq = spool.tile([P, 20], F32, tag="sq")
                nc.scalar.activation(sq[:], prod[:],
                                     mybir.ActivationFunctionType.Sqrt)
                mx = spool.tile([P, 20], F32, tag="mx")
                nc.vector.tensor_scalar_max(mx[:], sq[:], EPS)
                rs = spool.tile([P, 20], F32, tag="rs")
                nc.vector.reciprocal(rs[:], mx[:])
                simt = spool.tile([P, 20], F32, tag="simt")
                nc.vector.tensor_tensor(
                    out=simt[:].rearrange("p (t n) -> p t n", t=4),
                    in0=rv[:, :, 0:5],
                    in1=rs[:].rearrange("p (t n) -> p t n", t=4), op=MULT)

                # --- weighted sum of raw shots. ACT scales shots 0/2/4 and
                # DVE (2x tensor_scalar) shot 3; PE accumulates those four
                # in PSUM via identity matmuls; DVE folds shot 1 with a
                # fused scale+add reading the PSUM partial ---
                for t in range(4):
                    def st(n):
                        return simt[:, t * 5 + n:t * 5 + n + 1]
                    a0 = wpool.tile([P, FREE], BF16, tag="a0")
                    nc.scalar.activation(
                        a0[:], xts[t][:, 0, :],
                        mybir.ActivationFunctionType.Copy, scale=st(0))
                    a2 = wpool.tile([P, FREE], BF16, tag="a2")
                    nc.scalar.activation(
                        a2[:], xts[t][:, 2, :],
                        mybir.ActivationFunctionType.Copy, scale=st(2))
                    a4 = wpool.tile([P, FREE], BF16, tag="a4")
                    nc.scalar.activation(
                        a4[:], xts[t][:, 4, :],
                        mybir.ActivationFunctionType.Copy, scale=st(4))
                    a3 = wpool.tile([P, FREE], BF16, tag="a3")
                    nc.vector.tensor_scalar(
                        out=a3[:], in0=xts[t][:, 3, :], scalar1=st(3),
                        scalar2=None, op0=MULT)
                    ea = eapool.tile([P, FREE], F32, tag="ea")
                    nc.tensor.matmul(ea[:], lhs_eye, a0[:],
                                     start=True, stop=False)
                    nc.tensor.matmul(ea[:], lhs_eye, a2[:],
                                     start=False, stop=False)
                    nc.tensor.matmul(ea[:], lhs_eye, a4[:],
                                     start=False, stop=False)
                    nc.tensor.matmul(ea[:], lhs_eye, a3[:],
                                     start=False, stop=True)
                    ob = wpool.tile([P, FREE], F32, tag="ob")
                    nc.vector.scalar_tensor_tensor(
                        out=ob[:], in0=xts[t][:, 1, :], scalar=st(1),
                        in1=ea[:], op0=MULT, op1=ADD)
                    nc.sync.dma_start(ov[4 * g + t], ob[:])

    nc.compile()
    return nc


_CACHE = {}


def _get_nc(bc: int = BC):
    if bc not in _CACHE:
        _CACHE[bc] = build(bc)
    return _CACHE[bc]


def kernel(x: np.ndarray) -> np.ndarray:
    assert x.shape == (B, N, C, 10, 10) and x.dtype == np.float32
    nc = _get_nc(BC)
    cs = consts_np()
    shards = np.ascontiguousarray(x.reshape(NCORES, BC, N, C, HW))
    in_maps = [{"x": shards[i], "consts": cs} for i in range(NCORES)]
    res = run_bass_kernel_spmd(nc, in_maps, core_ids=list(range(NCORES)))
    out = np.concatenate([res.results[i]["out"] for i in range(NCORES)])
    return out.reshape(B, C, 10, 10).astype(np.float32)

